# revision 5
# baseline (speedup 1.0000x reference)
"""Trainium2 Bass kernel for nn_Decoder (2-layer LSTM decoder + VAE heads).

Math (mirrors the jax reference exactly, including its arg-swap quirk):
    z_cat = concat([z1, z2], -1)                      [bs, 64]
    xw1   = z_cat @ W1 + b1                           (constant per step)
    per step t (T=20):
        g1 = xw1 + h1 @ U1          ; h1,c1 = lstm_gates(g1, c1)
        g2 = h1n @ W2 + b2 + h2 @ U2; h2,c2 = lstm_gates(g2, c2)
    output   = stack of h2                            [bs, T, 256]
    x_mu     = output @ Wmu + bmu
    x_logvar = output @ Wlv + blv
    x_sample = eps * exp(0.5 * x_mu) + x_logvar

Distribution: pure data parallel — batch 8192 split as 1024 rows/core over
8 NeuronCores; weights replicated; the T=20 scan is local per core.

Device layout is "transposed": activations are [feature, batch] so that
every matmul consumes weights in their natural [K, M] (lhsT) layout and no
on-device transposes are needed.  Per core the batch (1024) is processed
in NCH chunks of NB columns which are software-pipelined across the step
loop to keep all engines busy.  Host side does the cheap transposes.

b1 is folded into the W1 matmul via an appended ones-row in z_cat.
b2 is applied via per-partition ACT bias only when nonzero (the problem
spec fills it with zeros, which keeps the fast single-call ACT path).
"""

import os
import sys
import numpy as np

if "/opt/trn_rl_repo" not in sys.path:  # harmless if axon site already provides it
    sys.path.append("/opt/trn_rl_repo")

import concourse.bacc as bacc
import concourse.tile as tile
from concourse import mybir
from concourse.bass_utils import run_bass_kernel_spmd

F32 = mybir.dt.float32
F32R = mybir.dt.float32r
AF = mybir.ActivationFunctionType

BS, T, FDIM, ZDIM, H = 8192, 20, 80, 32, 256
NCORES = 8
B = BS // NCORES  # 1024 batch rows per core
NB = int(os.environ.get("LSTM_NB", "512"))  # batch-chunk width per core
NCH = B // NB
G4 = 4 * H  # 1024 gate columns


def _build(nb, nch, t_steps, bias2, bias_mu, bias_lv):
    """Emit + compile the per-core program. bias2/bias_mu/bias_lv are flags
    for whether the (spec-zero) biases need to be applied on device."""
    nc = bacc.Bacc(
        "TRN2", target_bir_lowering=False, debug=False, num_devices=NCORES
    )

    ZC = nc.dram_tensor("zc", [2 * ZDIM + 1, B], F32, kind="ExternalInput").ap()
    EPS = nc.dram_tensor("eps", [t_steps, FDIM, B], F32, kind="ExternalInput").ap()
    W1A = nc.dram_tensor("w1a", [2 * ZDIM + 1, G4], F32, kind="ExternalInput").ap()
    U1 = nc.dram_tensor("u1", [H, G4], F32, kind="ExternalInput").ap()
    W2 = nc.dram_tensor("w2", [H, G4], F32, kind="ExternalInput").ap()
    U2 = nc.dram_tensor("u2", [H, G4], F32, kind="ExternalInput").ap()
    WHD = nc.dram_tensor("whd", [H, 2 * FDIM], F32, kind="ExternalInput").ap()
    if bias2:
        B2 = nc.dram_tensor("b2", [G4], F32, kind="ExternalInput").ap()
    if bias_mu or bias_lv:
        BHD = nc.dram_tensor("bhd", [3, FDIM], F32, kind="ExternalInput").ap()

    HT = nc.dram_tensor("ht", [t_steps, H, B], F32, kind="ExternalOutput").ap()
    MUT = nc.dram_tensor("mut", [t_steps, FDIM, B], F32, kind="ExternalOutput").ap()
    LVT = nc.dram_tensor("lvt", [t_steps, FDIM, B], F32, kind="ExternalOutput").ap()
    ST = nc.dram_tensor("st", [t_steps, FDIM, B], F32, kind="ExternalOutput").ap()

    n2 = 2 * nb

    with tile.TileContext(nc) as tc:
        with (
            tc.tile_pool(name="stg", bufs=2) as stg,
            tc.tile_pool(name="wt", bufs=1) as wt,
            tc.tile_pool(name="state", bufs=1) as state,
            tc.tile_pool(name="ps", bufs=2, space="PSUM") as ps,
            tc.tile_pool(name="sig", bufs=2) as sigp,
            tc.tile_pool(name="tmp", bufs=2) as tmpp,
            tc.tile_pool(name="hd", bufs=3) as hdp,
            tc.tile_pool(name="epi", bufs=3) as epip,
        ):
            # ---- one-time: load weights, round to fp32r ----
            def load_f32r(src_ap, shape, nm):
                f = stg.tile(shape, F32, tag="stage", name=f"stg_{nm}")
                nc.sync.dma_start(out=f[:], in_=src_ap)
                r = wt.tile(shape, F32R, name=f"r_{nm}")
                nc.vector.tensor_copy(r[:], f[:])
                return r

            w1r = load_f32r(W1A, [2 * ZDIM + 1, G4], "w1")
            u1r = [load_f32r(U1[k * 128 : (k + 1) * 128, :], [128, G4], f"u1{k}") for k in range(2)]
            w2r = [load_f32r(W2[k * 128 : (k + 1) * 128, :], [128, G4], f"w2{k}") for k in range(2)]
            u2r = [load_f32r(U2[k * 128 : (k + 1) * 128, :], [128, G4], f"u2{k}") for k in range(2)]
            whr = [load_f32r(WHD[k * 128 : (k + 1) * 128, :], [128, 2 * FDIM], f"wh{k}") for k in range(2)]
            zcr = load_f32r(ZC, [2 * ZDIM + 1, B], "zc")

            b2t = None
            if bias2:
                b2t = wt.tile([128, 8], F32, name="b2t")
                for m in range(8):
                    nc.sync.dma_start(out=b2t[:, m : m + 1], in_=B2[m * 128 : (m + 1) * 128])
            bmu_t = blv_t = bmu2_t = None
            if bias_mu or bias_lv:
                bh = wt.tile([FDIM, 3], F32, name="bht")
                # BHD rows: [bmu, blv, 0.5*bmu] -> columns of bh
                for r in range(3):
                    nc.sync.dma_start(out=bh[:, r : r + 1], in_=BHD[r, :])
                bmu_t, blv_t, bmu2_t = bh[:, 0:1], bh[:, 1:2], bh[:, 2:3]

            # ---- persistent per-chunk state ----
            h1 = [[state.tile([128, nb], F32R, name=f"h1_{n}_{j}") for j in range(2)] for n in range(nch)]
            h2 = [[state.tile([128, nb], F32R, name=f"h2_{n}_{j}") for j in range(2)] for n in range(nch)]
            c1 = [state.tile([128, n2], F32, name=f"c1_{n}") for n in range(nch)]
            c2 = [state.tile([128, n2], F32, name=f"c2_{n}") for n in range(nch)]

            def lstm_layer(t, n, lname, in_pairs, rec_w, rec_h):
                """Emit the gate matmuls for one LSTM layer of (step t, chunk n).
                in_pairs: [(weight_tile, rhs_tile_or_slice)] input projections;
                rec_w/rec_h: recurrent weight k-tiles + state rhs (skipped at t=0,
                when the state is still zero)."""
                A = ps.tile([128, 4 * nb], F32, tag="ps", name=f"A_{lname}_{t}_{n}")
                Bp = ps.tile([128, 4 * nb], F32, tag="ps", name=f"B_{lname}_{t}_{n}")
                for half, pt in ((0, A), (1, Bp)):
                    for mi in range(4):
                        m = half * 4 + mi
                        sl = pt[:, mi * nb : (mi + 1) * nb]
                        mm = [(w[:, m * 128 : (m + 1) * 128], rhs) for (w, rhs) in in_pairs]
                        if t > 0:
                            mm += [
                                (rec_w[k][:, m * 128 : (m + 1) * 128], rec_h[k][:])
                                for k in range(2)
                            ]
                        for i, (lhsT, rhs) in enumerate(mm):
                            nc.tensor.matmul(
                                sl, lhsT, rhs, start=(i == 0), stop=(i == len(mm) - 1)
                            )
                return A, Bp

            def lstm_elem(t, n, A, Bp, hst, cst, lname, b2sl):
                """sigmoid/tanh + cell update for one layer."""
                sif = sigp.tile([128, 4 * nb], F32, tag="sif", name=f"sif_{lname}_{t}_{n}")
                if b2sl is None:
                    nc.scalar.activation(sif[:, 0 : 4 * nb], A[:, 0 : 4 * nb], AF.Sigmoid)
                else:
                    for mi in range(4):
                        nc.scalar.activation(
                            sif[:, mi * nb : (mi + 1) * nb],
                            A[:, mi * nb : (mi + 1) * nb],
                            AF.Sigmoid,
                            bias=b2sl[mi],
                        )
                tg = sigp.tile([128, n2], F32, tag="tg", name=f"tg_{lname}_{t}_{n}")
                so = sigp.tile([128, n2], F32, tag="so", name=f"so_{lname}_{t}_{n}")
                if b2sl is None:
                    nc.scalar.activation(tg[:], Bp[:, 0:n2], AF.Tanh)
                    nc.scalar.activation(so[:], Bp[:, n2 : 4 * nb], AF.Sigmoid)
                else:
                    for mi in range(2):
                        nc.scalar.activation(
                            tg[:, mi * nb : (mi + 1) * nb],
                            Bp[:, mi * nb : (mi + 1) * nb],
                            AF.Tanh,
                            bias=b2sl[4 + mi],
                        )
                        nc.scalar.activation(
                            so[:, mi * nb : (mi + 1) * nb],
                            Bp[:, n2 + mi * nb : n2 + (mi + 1) * nb],
                            AF.Sigmoid,
                            bias=b2sl[6 + mi],
                        )
                if t == 0:
                    # c = sigmoid(i) * tanh(g)
                    nc.vector.tensor_mul(cst[:], sif[:, 0:n2], tg[:])
                else:
                    t1 = tmpp.tile([128, n2], F32, tag="t1", name=f"t1_{lname}_{t}_{n}")
                    u = tmpp.tile([128, n2], F32, tag="u", name=f"u_{lname}_{t}_{n}")
                    nc.vector.tensor_mul(t1[:], sif[:, 0:n2], tg[:])
                    nc.gpsimd.tensor_mul(u[:], sif[:, n2 : 4 * nb], cst[:])
                    nc.vector.tensor_add(cst[:], u[:], t1[:])
                tnc = tmpp.tile([128, n2], F32, tag="tnc", name=f"tnc_{lname}_{t}_{n}")
                nc.scalar.activation(tnc[:], cst[:], AF.Tanh)
                for j in range(2):
                    nc.vector.tensor_mul(
                        hst[j][:], so[:, j * nb : (j + 1) * nb], tnc[:, j * nb : (j + 1) * nb]
                    )

            b2sl_l2 = [b2t[:, m : m + 1] for m in range(8)] if bias2 else None

            for t in range(t_steps):
                for n in range(nch):
                    zsl = zcr[:, n * nb : (n + 1) * nb]
                    # ---- layer 1 (b1 already folded into w1r's ones-row) ----
                    A, Bp = lstm_layer(t, n, "l1", [(w1r, zsl)], u1r, h1[n])
                    lstm_elem(t, n, A, Bp, h1[n], c1[n], "l1", None)
                    # ---- layer 2 ----
                    A2, B2p = lstm_layer(
                        t, n, "l2",
                        [(w2r[0], h1[n][0][:]), (w2r[1], h1[n][1][:])],
                        u2r, h2[n],
                    )
                    lstm_elem(t, n, A2, B2p, h2[n], c2[n], "l2", b2sl_l2)
                    # ---- heads ----
                    Hp = ps.tile([FDIM, n2], F32, tag="ps", name=f"H_{t}_{n}")
                    for col, off in ((0, 0), (1, FDIM)):
                        for k in range(2):
                            nc.tensor.matmul(
                                Hp[:, col * nb : (col + 1) * nb],
                                whr[k][:, off : off + FDIM],
                                h2[n][k][:],
                                start=(k == 0),
                                stop=(k == 1),
                            )
                    E = hdp.tile([FDIM, nb], F32, tag="E", name=f"E_{t}_{n}")
                    if bias_mu:
                        nc.scalar.activation(E[:], Hp[:, 0:nb], AF.Exp, scale=0.5, bias=bmu2_t)
                    else:
                        nc.scalar.activation(E[:], Hp[:, 0:nb], AF.Exp, scale=0.5)
                    mlv = hdp.tile([FDIM, n2], F32, tag="mlv", name=f"mlv_{t}_{n}")
                    if bias_mu or bias_lv:
                        nc.scalar.activation(mlv[:, 0:nb], Hp[:, 0:nb], AF.Identity, bias=bmu_t)
                        nc.scalar.activation(mlv[:, nb:n2], Hp[:, nb:n2], AF.Identity, bias=blv_t)
                    else:
                        nc.scalar.activation(mlv[:], Hp[:], AF.Copy)
                    ep = epip.tile([FDIM, nb], F32, tag="ep", name=f"ep_{t}_{n}")
                    nc.sync.dma_start(out=ep[:], in_=EPS[t, :, n * nb : (n + 1) * nb])
                    sm = epip.tile([FDIM, nb], F32, tag="sm", name=f"sm_{t}_{n}")
                    nc.gpsimd.tensor_mul(sm[:], ep[:], E[:])
                    ss = epip.tile([FDIM, nb], F32, tag="ss", name=f"ss_{t}_{n}")
                    nc.vector.tensor_add(ss[:], sm[:], mlv[:, nb:n2])
                    # ---- stores ----
                    zr = slice(n * nb, (n + 1) * nb)
                    for j in range(2):
                        nc.sync.dma_start(
                            out=HT[t, j * 128 : (j + 1) * 128, zr],
                            in_=h2[n][j][:].bitcast(F32),
                        )
                    nc.sync.dma_start(out=MUT[t, :, zr], in_=mlv[:, 0:nb])
                    nc.sync.dma_start(out=LVT[t, :, zr], in_=mlv[:, nb:n2])
                    nc.sync.dma_start(out=ST[t, :, zr], in_=ss[:])

    nc.compile()
    return nc


_cache = {}


def _get_program(key):
    if key not in _cache:
        _cache[key] = _build(*key)
    return _cache[key]


def run_full(inputs, trace=False, **spmd_kwargs):
    """Run the full problem on 8 cores.  Returns ((output, x_mu, x_logvar,
    x_sample), BassKernelResults)."""
    z1 = np.asarray(inputs["z1"], np.float32)
    z2 = np.asarray(inputs["z2"], np.float32)
    eps = np.asarray(inputs["eps"], np.float32)
    W1 = np.asarray(inputs["W1"], np.float32)
    U1 = np.asarray(inputs["U1"], np.float32)
    b1 = np.asarray(inputs["b1"], np.float32)
    W2 = np.asarray(inputs["W2"], np.float32)
    U2 = np.asarray(inputs["U2"], np.float32)
    b2 = np.asarray(inputs["b2"], np.float32)
    Wmu = np.asarray(inputs["Wmu"], np.float32)
    bmu = np.asarray(inputs["bmu"], np.float32)
    Wlv = np.asarray(inputs["Wlv"], np.float32)
    blv = np.asarray(inputs["blv"], np.float32)

    bias2 = bool(np.any(b2))
    bias_mu = bool(np.any(bmu))
    bias_lv = bool(np.any(blv))
    nc = _get_program((NB, NCH, T, bias2, bias_mu, bias_lv))

    w1a = np.vstack([W1, b1[None, :]])  # b1 folded via ones-row of z_cat
    whd = np.hstack([Wmu, Wlv])
    base = {"w1a": w1a, "u1": U1, "w2": W2, "u2": U2, "whd": whd}
    if bias2:
        base["b2"] = b2
    if bias_mu or bias_lv:
        base["bhd"] = np.stack([bmu, blv, 0.5 * bmu])

    in_maps = []
    for c in range(NCORES):
        rows = slice(c * B, (c + 1) * B)
        m = dict(base)
        m["zc"] = make_zc(z1[rows], z2[rows])
        m["eps"] = np.ascontiguousarray(eps[rows].transpose(1, 2, 0))
        in_maps.append(m)

    res = run_bass_kernel_spmd(
        nc, in_maps, list(range(NCORES)), trace=trace, **spmd_kwargs
    )

    output = np.empty((BS, T, H), np.float32)
    x_mu = np.empty((BS, T, FDIM), np.float32)
    x_lv = np.empty((BS, T, FDIM), np.float32)
    x_s = np.empty((BS, T, FDIM), np.float32)
    for c in range(NCORES):
        rows = slice(c * B, (c + 1) * B)
        r = res.results[c]
        output[rows] = r["ht"].transpose(2, 0, 1)
        x_mu[rows] = r["mut"].transpose(2, 0, 1)
        x_lv[rows] = r["lvt"].transpose(2, 0, 1)
        x_s[rows] = r["st"].transpose(2, 0, 1)
    return (output, x_mu, x_lv, x_s), res


def make_zc(z1r, z2r):
    zc = np.empty((2 * ZDIM + 1, z1r.shape[0]), np.float32)
    zc[0:ZDIM] = z1r.T
    zc[ZDIM : 2 * ZDIM] = z2r.T
    zc[2 * ZDIM] = 1.0
    return zc


def kernel(**inputs):
    return run_full(inputs, trace=False)[0]
